# revision 12
# baseline (speedup 1.0000x reference)
"""Local/global multihead attention on 8 NeuronCores (Trainium2, Bass/Tile).

Sharding: core c = b*2 + hg  (b = batch 0..3, hg = head-group 0/1, 8 heads each).
Each core computes q/k/v projections for its 8 heads on its batch, head-local
attention (slot 0 runs a dense 2048-key path driven by a per-core mask so the
SPMD program is uniform: hg0's slot 0 is the true global head with an all-ones
mask, hg1's slot 0 is a local head with a band mask), banded attention with
narrowed tq windows for slots 1-7, and the output projection restricted to its
head-group columns of wo. Host sums the two head-group partials per batch and
adds bo + bv @ wo.T (valid because softmax rows sum to 1).

Performance structure: s (token-quarter) outer loop; banded heads are
processed in even/odd pairs whose K=64 QK matmuls land in PE row-groups 0/64
(tile_position auto-derived from base_partition) and overlap on the 128x128
array; v-projection is emitted just-in-time inside slot0's first pass and the
fc1-3 q/k projections are interleaved between the first quarter's attention
blocks so ScalarE/DVE attention work overlaps PE projection work. Banded mask
multiplies are consolidated to one [128,2304] DVE op per (head, s) via a
block-layout scratch tile. The k bias is dropped (softmax shift invariance)
and the q bias rides the ScalarE PSUM evacuation. Softmax denominators use
reciprocal_approx_fast (SBUF-staged: the op mis-executes on PSUM inputs).

All matmul operands are bf16 (TensorE runs 1 cyc/row vs 4 for fp32); PSUM
accumulation is fp32 throughout.
"""
import numpy as np
import ml_dtypes

E, H, D, LK = 1024, 16, 64, 128
SCALE = D ** -0.5
B, N = 4, 2048
FG = 512          # features per head-group (8 heads * 64)
NCORES = 8

# narrowed tq windows per dj variant (delta = (dj-1)*128)
WIN = [(0, 128), (0, 256), (0, 384), (128, 512), (256, 512), (384, 512)]

_cache = {}


def _bf16(a):
    return np.ascontiguousarray(a.astype(ml_dtypes.bfloat16))


def _build():
    import concourse.bacc as bacc
    import concourse.tile as tile
    import concourse.mybir as mybir
    from concourse.bass import ts

    dt = mybir.dt
    AF = mybir.ActivationFunctionType

    nc = bacc.Bacc("TRN2", target_bir_lowering=False, debug=False,
                   num_devices=NCORES)

    xT = nc.dram_tensor("xT", [E, N], dt.bfloat16, kind="ExternalInput")
    wqT = nc.dram_tensor("wqT", [E, FG], dt.bfloat16, kind="ExternalInput")
    wkT = nc.dram_tensor("wkT", [E, FG], dt.bfloat16, kind="ExternalInput")
    wvT = nc.dram_tensor("wvT", [E, FG], dt.bfloat16, kind="ExternalInput")
    woT = nc.dram_tensor("woT", [FG, E], dt.bfloat16, kind="ExternalInput")
    bqc = nc.dram_tensor("bqc", [128, 4], dt.float32, kind="ExternalInput")
    # strip0 [128, 3968]: slot-0 mask table. slice at 512s-128jc+1920 gives the
    # [128,512] mask for (jc, s): all-ones on hg0 (global head), band on hg1.
    strip0 = nc.dram_tensor("strip0", [128, 3968], dt.bfloat16, kind="ExternalInput")
    # supb [128, 2304]: banded mask blocks; block dj at cols [384dj, 384dj+w).
    supb = nc.dram_tensor("supb", [128, 2304], dt.bfloat16, kind="ExternalInput")
    out = nc.dram_tensor("out", [N, E], dt.float32, kind="ExternalOutput")

    with tile.TileContext(nc) as tc:
        with (
            tc.tile_pool(name="wts", bufs=1) as wts,
            tc.tile_pool(name="xp", bufs=1) as xp,
            tc.tile_pool(name="qkv", bufs=1) as qkv,
            tc.tile_pool(name="att", bufs=3) as att,
            tc.tile_pool(name="sup", bufs=2) as sup,
            tc.tile_pool(name="small", bufs=4) as small,
            tc.tile_pool(name="ps", bufs=4, space="PSUM") as psp,
            tc.tile_pool(name="av", bufs=1, space="PSUM") as avp,
        ):
            # ---- load weights/x/masks ----
            xT_t = [xp.tile([128, N], dt.bfloat16, name=f"xT{i}", tag=f"xT{i}") for i in range(8)]
            for ec in range(8):
                nc.sync.dma_start(xT_t[ec][:], xT[ts(ec, 128), :])
            wq_t = [wts.tile([128, FG], dt.bfloat16, name=f"wq{i}", tag=f"wq{i}") for i in range(8)]
            wk_t = [wts.tile([128, FG], dt.bfloat16, name=f"wk{i}", tag=f"wk{i}") for i in range(8)]
            wv_t = [wts.tile([128, FG], dt.bfloat16, name=f"wv{i}", tag=f"wv{i}") for i in range(8)]
            for ec in range(8):
                nc.sync.dma_start(wq_t[ec][:], wqT[ts(ec, 128), :])
                nc.sync.dma_start(wk_t[ec][:], wkT[ts(ec, 128), :])
                nc.sync.dma_start(wv_t[ec][:], wvT[ts(ec, 128), :])
            wo_t = [wts.tile([128, E], dt.bfloat16, name=f"wo{i}", tag=f"wo{i}") for i in range(4)]
            for fc in range(4):
                nc.sync.dma_start(wo_t[fc][:], woT[ts(fc, 128), :])
            bq_t = small.tile([128, 4], dt.float32, name="bq", tag="bq")
            nc.sync.dma_start(bq_t[:], bqc[:, :])
            s0_t = wts.tile([128, 3968], dt.bfloat16, name="s0", tag="s0")
            nc.sync.dma_start(s0_t[:], strip0[:, :])
            sup_t = wts.tile([128, 2304], dt.bfloat16, name="supb", tag="supb")
            nc.sync.dma_start(sup_t[:], supb[:, :])

            qT_sb = [qkv.tile([128, N], dt.bfloat16, name=f"qT{i}", tag=f"qT{i}") for i in range(4)]
            kT_sb = [qkv.tile([128, N], dt.bfloat16, name=f"kT{i}", tag=f"kT{i}") for i in range(4)]
            v_sb = [qkv.tile([128, 8 * 72], dt.bfloat16, name=f"v{i}", tag=f"v{i}") for i in range(16)]
            outTn = [qkv.tile([128, N], dt.bfloat16, name=f"outTn{i}", tag=f"outTn{i}") for i in range(4)]

            # k bias dropped (softmax shift invariance); q bias rides the
            # ScalarE evacuation as a per-partition activation bias.
            def emit_qk(fc):
                for dst, w_t, biased in ((qT_sb, wq_t, True), (kT_sb, wk_t, False)):
                    for tcn in range(4):
                        ps = psp.tile([128, 512], dt.float32, name="ps", tag="ps")
                        for ec in range(8):
                            nc.tensor.matmul(
                                ps[:], w_t[ec][:, ts(fc, 128)],
                                xT_t[ec][:, ts(tcn, 512)],
                                start=(ec == 0), stop=(ec == 7))
                        if biased:
                            nc.scalar.activation(
                                dst[fc][:, ts(tcn, 512)], ps[:], AF.Identity,
                                bias=bq_t[:, fc:fc + 1])
                        else:
                            nc.scalar.copy(dst[fc][:, ts(tcn, 512)], ps[:])

            def emit_v(tcn):
                # v natural layout, per-head 72-col strided tiles w/ ones col
                ps = psp.tile([128, 512], dt.float32, name="ps", tag="ps")
                for ec in range(8):
                    nc.tensor.matmul(ps[:], xT_t[ec][:, ts(tcn, 128)], wv_t[ec][:],
                                     start=(ec == 0), stop=(ec == 7))
                src = ps[:].rearrange("p (h d) -> p h d", h=8)
                dst = v_sb[tcn][:].rearrange("p (h d) -> p h d", h=8)[:, :, 0:64]
                nc.vector.tensor_copy(dst, src)
                ones = v_sb[tcn][:].rearrange("p (h d) -> p h d", h=8)[:, :, 64:65]
                nc.vector.memset(ones, 1.0)

            def head_rows(t, h):
                r0 = (h % 2) * 64
                return t[h // 2][r0:r0 + 64, :]

            def norm(h, s, av):
                # den -> SBUF (approx recip mis-executes on PSUM), recip,
                # broadcast to 64 partitions, scale the head's outTn rows.
                den = small.tile([1, 512], dt.float32, name="den", tag="den")
                nc.scalar.copy(den[:], av[64:65, :])
                rec = small.tile([1, 512], dt.float32, name="rec", tag="rec")
                nc.vector.reciprocal_approx_fast(rec[:], den[:])
                rec64 = small.tile([64, 512], dt.float32, name="rec64", tag="rec64")
                nc.gpsimd.partition_broadcast(rec64[:], rec[:])
                nc.vector.tensor_mul(head_rows(outTn, h)[:, ts(s, 512)],
                                     av[0:64, :], rec64[:])

            def banded(h, s, avtag, abuf):
                # one banded head for token-quarter s: QK per window into the
                # block-layout at_super, exp per window, ONE mask multiply,
                # then AV accumulation per window.
                qh = head_rows(qT_sb, h)
                kh = head_rows(kT_sb, h)
                djs = [dj for dj in range(6) if 0 <= s * 4 - 1 + dj <= 15]
                ats = sup.tile([128, 2304], dt.bfloat16, name="ats", tag=avtag + "ats")
                pss = []
                for dj in djs:
                    jc = s * 4 - 1 + dj
                    c0, c1 = WIN[dj]
                    ps = psp.tile([128, 512], dt.float32, name="ps", tag="ps")
                    nc.tensor.matmul(ps[:, c0:c1], kh[:, ts(jc, 128)],
                                     qh[:, s * 512 + c0:s * 512 + c1],
                                     start=True, stop=True)
                    pss.append(ps)
                for dj, ps in zip(djs, pss):
                    c0, c1 = WIN[dj]
                    nc.scalar.activation(ats[:, 384 * dj:384 * dj + (c1 - c0)],
                                         ps[:, c0:c1], AF.Exp, scale=float(SCALE))
                nc.vector.tensor_mul(ats[:], ats[:], sup_t[:])
                av = avp.tile([128, 512], dt.float32, name="av", tag=avtag, bufs=abuf)
                for i, dj in enumerate(djs):
                    jc = s * 4 - 1 + dj
                    c0, c1 = WIN[dj]
                    nc.tensor.matmul(
                        av[0:65, c0:c1], v_sb[jc][:, h * 72:h * 72 + 65],
                        ats[:, 384 * dj:384 * dj + (c1 - c0)],
                        start=(i == 0), stop=(i == len(djs) - 1),
                        skip_group_check=True)
                norm(h, s, av)

            def banded_pair(ha, hb, s):
                # even/odd head pair: QK matmuls alternate PE row-groups 0/64
                # (auto tile_position) so the K=64 matmuls run concurrently.
                djs = [dj for dj in range(6) if 0 <= s * 4 - 1 + dj <= 15]
                tiles = {}
                for h, tag in ((ha, "ava"), (hb, "avb")):
                    tiles[h] = (sup.tile([128, 2304], dt.bfloat16, name="ats",
                                         tag=tag + "ats"), [])
                for dj in djs:
                    jc = s * 4 - 1 + dj
                    c0, c1 = WIN[dj]
                    for h in (ha, hb):
                        qh = head_rows(qT_sb, h)
                        kh = head_rows(kT_sb, h)
                        ps = psp.tile([128, 512], dt.float32, name="ps", tag="ps")
                        nc.tensor.matmul(ps[:, c0:c1], kh[:, ts(jc, 128)],
                                         qh[:, s * 512 + c0:s * 512 + c1],
                                         start=True, stop=True)
                        tiles[h][1].append(ps)
                for h in (ha, hb):
                    ats, pss = tiles[h]
                    for dj, ps in zip(djs, pss):
                        c0, c1 = WIN[dj]
                        nc.scalar.activation(ats[:, 384 * dj:384 * dj + (c1 - c0)],
                                             ps[:, c0:c1], AF.Exp,
                                             scale=float(SCALE))
                    nc.vector.tensor_mul(ats[:], ats[:], sup_t[:])
                avs = {}
                for h, tag in ((ha, "ava"), (hb, "avb")):
                    avs[h] = avp.tile([128, 512], dt.float32, name="av",
                                      tag=tag, bufs=1)
                for i, dj in enumerate(djs):
                    jc = s * 4 - 1 + dj
                    c0, c1 = WIN[dj]
                    for h in (ha, hb):
                        nc.tensor.matmul(
                            avs[h][0:65, c0:c1],
                            v_sb[jc][:, h * 72:h * 72 + 65],
                            tiles[h][0][:, 384 * dj:384 * dj + (c1 - c0)],
                            start=(i == 0), stop=(i == len(djs) - 1),
                            skip_group_check=True)
                norm(ha, s, avs[ha])
                norm(hb, s, avs[hb])

            def outproj(s):
                for tcn in range(4 * s, 4 * s + 4):
                    for oc in range(2):
                        ps = psp.tile([128, 512], dt.float32, name="ps", tag="ps")
                        for fc in range(4):
                            nc.tensor.matmul(ps[:], outTn[fc][:, ts(tcn, 128)],
                                             wo_t[fc][:, ts(oc, 512)],
                                             start=(fc == 0), stop=(fc == 3))
                        ob = att.tile([128, 512], dt.float32, name="ob", tag="ob")
                        if oc == 0:
                            nc.scalar.copy(ob[:], ps[:])
                        else:
                            nc.vector.tensor_copy(ob[:], ps[:])
                        nc.sync.dma_start(out[ts(tcn, 128), ts(oc, 512)], ob[:])

            # ---- main schedule ----
            emit_qk(0)
            qh0 = head_rows(qT_sb, 0)
            kh0 = head_rows(kT_sb, 0)
            for s in range(4):
                # slot 0: dense 2048-key path; mask = static strip0 slices.
                # v projection is emitted just-in-time on the first pass.
                avg = avp.tile([128, 512], dt.float32, name="avg", tag="avg",
                               bufs=1)
                for jc in range(16):
                    if s == 0:
                        emit_v(jc)
                    ps = psp.tile([128, 512], dt.float32, name="ps", tag="ps")
                    nc.tensor.matmul(ps[:], kh0[:, ts(jc, 128)],
                                     qh0[:, ts(s, 512)], start=True, stop=True)
                    at = att.tile([128, 512], dt.bfloat16, name="at", tag="at")
                    nc.scalar.activation(at[:], ps[:], AF.Exp, scale=float(SCALE))
                    off = 512 * s - 128 * jc + 1920
                    eng = nc.gpsimd if jc % 2 else nc.vector
                    eng.tensor_mul(at[:], at[:], s0_t[:, off:off + 512])
                    nc.tensor.matmul(
                        avg[0:65, :], v_sb[jc][:, 0:65],
                        at[:], start=(jc == 0), stop=(jc == 15),
                        skip_group_check=True)
                norm(0, s, avg)
                banded(1, s, "av1", 1)
                for pi, (ha, hb) in enumerate(((2, 3), (4, 5), (6, 7))):
                    if s == 0:
                        emit_qk(pi + 1)
                    banded_pair(ha, hb, s)
                outproj(s)
    nc.finalize()
    return nc


def _host_inputs(x, wq, bq, wk, bk, wv, bv, wo, bo):
    """Build the 8 per-core input dicts."""
    r = np.arange(128)[:, None]
    # strip0 [128, 3968]: band at |r + 1920 - c| <= LK (hg1) or all ones (hg0)
    c = np.arange(3968)[None, :]
    strip0_band = _bf16((np.abs(r + 1920 - c) <= LK).astype(np.float32))
    strip0_ones = _bf16(np.ones((128, 3968), np.float32))
    # supb [128, 2304]: block dj holds the window-(dj) band mask at 384*dj
    supb = np.zeros((128, 2304), np.float32)
    for dj in range(6):
        c0, c1 = WIN[dj]
        cc = np.arange(c0, c1)[None, :]
        supb[:, 384 * dj:384 * dj + (c1 - c0)] = (
            np.abs((dj - 1) * 128 + r - cc) <= LK)
    supb = _bf16(supb)

    in_maps = []
    for core in range(NCORES):
        b, hg = core // 2, core % 2
        fsl = slice(hg * FG, (hg + 1) * FG)
        in_maps.append({
            "xT": _bf16(x[b].T),
            "wqT": _bf16(wq[fsl].T),
            "wkT": _bf16(wk[fsl].T),
            "wvT": _bf16(wv[fsl].T),
            "woT": _bf16(wo[:, fsl].T),
            "bqc": np.ascontiguousarray(bq[fsl].reshape(4, 128).T, np.float32),
            "strip0": strip0_ones if hg == 0 else strip0_band,
            "supb": supb,
        })
    return in_maps


def kernel(x, wq, bq, wk, bk, wv, bv, wo, bo):
    from concourse.bass_utils import run_bass_kernel_spmd

    x, wq, bq, wk, bk, wv, bv, wo, bo = (
        np.asarray(a, np.float32) for a in (x, wq, bq, wk, bk, wv, bv, wo, bo))

    if "nc" not in _cache:
        _cache["nc"] = _build()
    nc = _cache["nc"]

    in_maps = _host_inputs(x, wq, bq, wk, bk, wv, bv, wo, bo)
    res = run_bass_kernel_spmd(nc, in_maps, core_ids=list(range(NCORES)))
    _cache["last_results"] = res

    const = (bo + bv @ wo.T).astype(np.float32)        # [1024]
    out = np.empty((B, N, E), np.float32)
    for b in range(B):
        out[b] = res.results[2 * b]["out"] + res.results[2 * b + 1]["out"] + const
    return out


# revision 17
# speedup vs baseline: 1.1587x; 1.1587x over previous
"""Local/global multihead attention on 8 NeuronCores (Trainium2, Bass/Tile).

Sharding: core c = b*2 + hg  (b = batch 0..3, hg = head-group 0/1, 8 heads each).
Each core computes q/k/v projections for its 8 heads on its batch, head-local
attention (slot 0 runs a dense 2048-key path driven by a per-core mask so the
SPMD program is uniform: hg0's slot 0 is the true global head with an all-ones
mask, hg1's slot 0 is a local head with a band mask), banded attention with
narrowed tq windows for slots 1-7, and the output projection restricted to its
head-group columns of wo. Host sums the two head-group partials per batch and
adds bo + bv @ wo.T (valid because softmax rows sum to 1).

Performance structure: s (token-quarter) outer loop; banded heads are
processed in even/odd pairs whose K=64 QK matmuls land in PE row-groups 0/64
(tile_position auto-derived from base_partition) and overlap on the 128x128
array; v-projection is emitted just-in-time inside slot0's first pass and the
fc1-3 q/k projections are interleaved between the first quarter's attention
blocks so ScalarE/DVE attention work overlaps PE projection work. Banded mask
multiplies are consolidated to one [128,2304] DVE op per (head, s) via a
block-layout scratch tile. The k bias is dropped (softmax shift invariance)
and the q bias rides the ScalarE PSUM evacuation. Softmax denominators use
reciprocal_approx_fast (SBUF-staged: the op mis-executes on PSUM inputs).

All matmul operands are bf16 (TensorE runs 1 cyc/row vs 4 for fp32); PSUM
accumulation is fp32 throughout.
"""
import numpy as np
import ml_dtypes

E, H, D, LK = 1024, 16, 64, 128
SCALE = D ** -0.5
B, N = 4, 2048
FG = 512          # features per head-group (8 heads * 64)
NCORES = 8

# narrowed tq windows per dj variant (delta = (dj-1)*128)
WIN = [(0, 128), (0, 256), (0, 384), (128, 512), (256, 512), (384, 512)]

_cache = {}


def _bf16(a):
    return np.ascontiguousarray(a.astype(ml_dtypes.bfloat16))


def _build():
    import concourse.bacc as bacc
    import concourse.tile as tile
    import concourse.mybir as mybir
    from concourse.bass import ts

    dt = mybir.dt
    AF = mybir.ActivationFunctionType

    nc = bacc.Bacc("TRN2", target_bir_lowering=False, debug=False,
                   num_devices=NCORES)

    xT = nc.dram_tensor("xT", [E, N], dt.bfloat16, kind="ExternalInput")
    wqT = nc.dram_tensor("wqT", [E, FG], dt.bfloat16, kind="ExternalInput")
    wkT = nc.dram_tensor("wkT", [E, FG], dt.bfloat16, kind="ExternalInput")
    wvT = nc.dram_tensor("wvT", [E, FG], dt.bfloat16, kind="ExternalInput")
    woT = nc.dram_tensor("woT", [FG, E], dt.bfloat16, kind="ExternalInput")
    bqc = nc.dram_tensor("bqc", [128, 4], dt.float32, kind="ExternalInput")
    # strip0 [128, 3968]: slot-0 mask table. slice at 512s-128jc+1920 gives the
    # [128,512] mask for (jc, s): all-ones on hg0 (global head), band on hg1.
    strip0 = nc.dram_tensor("strip0", [128, 3968], dt.bfloat16, kind="ExternalInput")
    # supb [128, 2304]: banded mask blocks; block dj at cols [384dj, 384dj+w).
    supb = nc.dram_tensor("supb", [128, 2304], dt.bfloat16, kind="ExternalInput")
    out = nc.dram_tensor("out", [N, E], dt.float32, kind="ExternalOutput")

    with tile.TileContext(nc) as tc:
        with (
            tc.tile_pool(name="wts", bufs=1) as wts,
            tc.tile_pool(name="xp", bufs=1) as xp,
            tc.tile_pool(name="qkv", bufs=1) as qkv,
            tc.tile_pool(name="att", bufs=3) as att,
            tc.tile_pool(name="sup", bufs=2) as sup,
            tc.tile_pool(name="small", bufs=4) as small,
            tc.tile_pool(name="ps", bufs=6, space="PSUM") as psp,
            tc.tile_pool(name="av", bufs=2, space="PSUM") as avp,
        ):
            # ---- load weights/x/masks ----
            xT_t = [xp.tile([128, N], dt.bfloat16, name=f"xT{i}", tag=f"xT{i}") for i in range(8)]
            for ec in range(8):
                nc.sync.dma_start(xT_t[ec][:], xT[ts(ec, 128), :])
            wq_t = [wts.tile([128, FG], dt.bfloat16, name=f"wq{i}", tag=f"wq{i}") for i in range(8)]
            wk_t = [wts.tile([128, FG], dt.bfloat16, name=f"wk{i}", tag=f"wk{i}") for i in range(8)]
            wv_t = [wts.tile([128, FG], dt.bfloat16, name=f"wv{i}", tag=f"wv{i}") for i in range(8)]
            for ec in range(8):
                nc.sync.dma_start(wq_t[ec][:], wqT[ts(ec, 128), :])
                nc.sync.dma_start(wk_t[ec][:], wkT[ts(ec, 128), :])
                nc.sync.dma_start(wv_t[ec][:], wvT[ts(ec, 128), :])
            wo_t = [wts.tile([128, E], dt.bfloat16, name=f"wo{i}", tag=f"wo{i}") for i in range(4)]
            for fc in range(4):
                nc.sync.dma_start(wo_t[fc][:], woT[ts(fc, 128), :])
            bq_t = small.tile([128, 4], dt.float32, name="bq", tag="bq")
            nc.sync.dma_start(bq_t[:], bqc[:, :])
            s0_t = wts.tile([128, 3968], dt.bfloat16, name="s0", tag="s0")
            nc.sync.dma_start(s0_t[:], strip0[:, :])
            sup_t = wts.tile([128, 2304], dt.bfloat16, name="supb", tag="supb")
            nc.sync.dma_start(sup_t[:], supb[:, :])

            qT_sb = [qkv.tile([128, N], dt.bfloat16, name=f"qT{i}", tag=f"qT{i}") for i in range(4)]
            kT_sb = [qkv.tile([128, N], dt.bfloat16, name=f"kT{i}", tag=f"kT{i}") for i in range(4)]
            v_sb = [qkv.tile([128, 8 * 72], dt.bfloat16, name=f"v{i}", tag=f"v{i}") for i in range(16)]
            outTn = [qkv.tile([128, N], dt.bfloat16, name=f"outTn{i}", tag=f"outTn{i}") for i in range(4)]

            # k bias dropped (softmax shift invariance); q bias rides the
            # ScalarE evacuation as a per-partition activation bias.
            def emit_qk(fc):
                for dst, w_t, biased in ((qT_sb, wq_t, True), (kT_sb, wk_t, False)):
                    for tcn in range(4):
                        ps = psp.tile([128, 512], dt.float32, name="ps", tag="ps")
                        for ec in range(8):
                            nc.tensor.matmul(
                                ps[:], w_t[ec][:, ts(fc, 128)],
                                xT_t[ec][:, ts(tcn, 512)],
                                start=(ec == 0), stop=(ec == 7))
                        if biased:
                            nc.scalar.activation(
                                dst[fc][:, ts(tcn, 512)], ps[:], AF.Identity,
                                bias=bq_t[:, fc:fc + 1])
                        else:
                            nc.scalar.copy(dst[fc][:, ts(tcn, 512)], ps[:])

            def emit_v(tcn):
                # v natural layout, per-head 72-col strided tiles w/ ones col
                ps = psp.tile([128, 512], dt.float32, name="ps", tag="ps")
                for ec in range(8):
                    nc.tensor.matmul(ps[:], xT_t[ec][:, ts(tcn, 128)], wv_t[ec][:],
                                     start=(ec == 0), stop=(ec == 7))
                src = ps[:].rearrange("p (h d) -> p h d", h=8)
                dst = v_sb[tcn][:].rearrange("p (h d) -> p h d", h=8)[:, :, 0:64]
                nc.vector.tensor_copy(dst, src)
                ones = v_sb[tcn][:].rearrange("p (h d) -> p h d", h=8)[:, :, 64:65]
                nc.vector.memset(ones, 1.0)

            def head_rows(t, h):
                r0 = (h % 2) * 64
                return t[h // 2][r0:r0 + 64, :]

            def norm(h, s, av):
                # den -> SBUF (approx recip mis-executes on PSUM), recip,
                # broadcast to 64 partitions, scale the head's outTn rows.
                den = small.tile([1, 512], dt.float32, name="den", tag="den")
                nc.scalar.copy(den[:], av[64:65, :])
                rec = small.tile([1, 512], dt.float32, name="rec", tag="rec")
                nc.vector.reciprocal_approx_fast(rec[:], den[:])
                rec64 = small.tile([64, 512], dt.float32, name="rec64", tag="rec64")
                nc.gpsimd.partition_broadcast(rec64[:], rec[:])
                nc.vector.tensor_mul(head_rows(outTn, h)[:, ts(s, 512)],
                                     av[0:64, :], rec64[:])

            def banded(h, s):
                # one banded head for token-quarter s: QK per window into the
                # block-layout at_super, exp per window, ONE mask multiply,
                # then AV accumulation per window.
                qh = head_rows(qT_sb, h)
                kh = head_rows(kT_sb, h)
                djs = [dj for dj in range(6) if 0 <= s * 4 - 1 + dj <= 15]
                ats = sup.tile([128, 2304], dt.bfloat16, name="ats", tag="aats")
                pss = []
                for dj in djs:
                    jc = s * 4 - 1 + dj
                    c0, c1 = WIN[dj]
                    ps = psp.tile([128, 512], dt.float32, name="ps", tag="ps")
                    nc.tensor.matmul(ps[:, c0:c1], kh[:, ts(jc, 128)],
                                     qh[:, s * 512 + c0:s * 512 + c1],
                                     start=True, stop=True)
                    pss.append(ps)
                for dj, ps in zip(djs, pss):
                    c0, c1 = WIN[dj]
                    nc.scalar.activation(ats[:, 384 * dj:384 * dj + (c1 - c0)],
                                         ps[:, c0:c1], AF.Exp, scale=float(SCALE))
                nc.vector.tensor_mul(ats[:], ats[:], sup_t[:])
                av = avp.tile([128, 512], dt.float32, name="av", tag="av")
                for i, dj in enumerate(djs):
                    jc = s * 4 - 1 + dj
                    c0, c1 = WIN[dj]
                    nc.tensor.matmul(
                        av[0:65, c0:c1], v_sb[jc][:, h * 72:h * 72 + 65],
                        ats[:, 384 * dj:384 * dj + (c1 - c0)],
                        start=(i == 0), stop=(i == len(djs) - 1),
                        skip_group_check=True)
                norm(h, s, av)

            def banded_pair(ha, hb, s):
                # even/odd head pair: QK matmuls alternate PE row-groups 0/64
                # (auto tile_position) so the K=64 matmuls run concurrently.
                djs = [dj for dj in range(6) if 0 <= s * 4 - 1 + dj <= 15]
                tiles = {}
                for h, tag in ((ha, "pa"), (hb, "pb")):
                    tiles[h] = (sup.tile([128, 2304], dt.bfloat16, name="ats",
                                         tag=tag + "ats"), [])
                for dj in djs:
                    jc = s * 4 - 1 + dj
                    c0, c1 = WIN[dj]
                    for h in (ha, hb):
                        qh = head_rows(qT_sb, h)
                        kh = head_rows(kT_sb, h)
                        ps = psp.tile([128, 512], dt.float32, name="ps", tag="ps")
                        nc.tensor.matmul(ps[:, c0:c1], kh[:, ts(jc, 128)],
                                         qh[:, s * 512 + c0:s * 512 + c1],
                                         start=True, stop=True)
                        tiles[h][1].append(ps)
                for h in (ha, hb):
                    ats, pss = tiles[h]
                    for dj, ps in zip(djs, pss):
                        c0, c1 = WIN[dj]
                        nc.scalar.activation(ats[:, 384 * dj:384 * dj + (c1 - c0)],
                                             ps[:, c0:c1], AF.Exp,
                                             scale=float(SCALE))
                    nc.vector.tensor_mul(ats[:], ats[:], sup_t[:])
                avs = {}
                for h in (ha, hb):
                    avs[h] = avp.tile([128, 512], dt.float32, name="av",
                                      tag="av")
                for i, dj in enumerate(djs):
                    jc = s * 4 - 1 + dj
                    c0, c1 = WIN[dj]
                    for h in (ha, hb):
                        nc.tensor.matmul(
                            avs[h][0:65, c0:c1],
                            v_sb[jc][:, h * 72:h * 72 + 65],
                            tiles[h][0][:, 384 * dj:384 * dj + (c1 - c0)],
                            start=(i == 0), stop=(i == len(djs) - 1),
                            skip_group_check=True)
                norm(ha, s, avs[ha])
                norm(hb, s, avs[hb])

            def outproj(s):
                for tcn in range(4 * s, 4 * s + 4):
                    for oc in range(2):
                        ps = psp.tile([128, 512], dt.float32, name="ps", tag="ps")
                        for fc in range(4):
                            nc.tensor.matmul(ps[:], outTn[fc][:, ts(tcn, 128)],
                                             wo_t[fc][:, ts(oc, 512)],
                                             start=(fc == 0), stop=(fc == 3))
                        ob = att.tile([128, 512], dt.float32, name="ob", tag="ob")
                        if oc == 0:
                            nc.scalar.copy(ob[:], ps[:])
                        else:
                            nc.vector.tensor_copy(ob[:], ps[:])
                        nc.sync.dma_start(out[ts(tcn, 128), ts(oc, 512)], ob[:])

            # ---- main schedule ----
            # outproj(s) is emitted inside block s+1 (deps long satisfied) so
            # its dense matmuls bridge the ScalarE-bound attention stretches.
            emit_qk(0)
            qh0 = head_rows(qT_sb, 0)
            kh0 = head_rows(kT_sb, 0)
            for s in range(4):
                # slot 0: dense 2048-key path; mask = static strip0 slices.
                # v projection is emitted just-in-time on the first pass.
                avg = avp.tile([128, 512], dt.float32, name="avg", tag="av")
                for jc in range(16):
                    if s == 0:
                        emit_v(jc)
                    ps = psp.tile([128, 512], dt.float32, name="ps", tag="ps")
                    nc.tensor.matmul(ps[:], kh0[:, ts(jc, 128)],
                                     qh0[:, ts(s, 512)], start=True, stop=True)
                    at = att.tile([128, 512], dt.bfloat16, name="at", tag="at")
                    nc.scalar.activation(at[:], ps[:], AF.Exp, scale=float(SCALE))
                    off = 512 * s - 128 * jc + 1920
                    nc.vector.tensor_mul(at[:], at[:], s0_t[:, off:off + 512])
                    nc.tensor.matmul(
                        avg[0:65, :], v_sb[jc][:, 0:65],
                        at[:], start=(jc == 0), stop=(jc == 15),
                        skip_group_check=True)
                norm(0, s, avg)
                if s > 0:
                    outproj(s - 1)
                banded(1, s)
                for pi, (ha, hb) in enumerate(((2, 3), (4, 5), (6, 7))):
                    if s == 0:
                        emit_qk(pi + 1)
                    banded_pair(ha, hb, s)
            outproj(3)
    nc.finalize()
    return nc


def _host_inputs(x, wq, bq, wk, bk, wv, bv, wo, bo):
    """Build the 8 per-core input dicts."""
    r = np.arange(128)[:, None]
    # strip0 [128, 3968]: band at |r + 1920 - c| <= LK (hg1) or all ones (hg0)
    c = np.arange(3968)[None, :]
    strip0_band = _bf16((np.abs(r + 1920 - c) <= LK).astype(np.float32))
    strip0_ones = _bf16(np.ones((128, 3968), np.float32))
    # supb [128, 2304]: block dj holds the window-(dj) band mask at 384*dj
    supb = np.zeros((128, 2304), np.float32)
    for dj in range(6):
        c0, c1 = WIN[dj]
        cc = np.arange(c0, c1)[None, :]
        supb[:, 384 * dj:384 * dj + (c1 - c0)] = (
            np.abs((dj - 1) * 128 + r - cc) <= LK)
    supb = _bf16(supb)

    in_maps = []
    for core in range(NCORES):
        b, hg = core // 2, core % 2
        fsl = slice(hg * FG, (hg + 1) * FG)
        in_maps.append({
            "xT": _bf16(x[b].T),
            "wqT": _bf16(wq[fsl].T),
            "wkT": _bf16(wk[fsl].T),
            "wvT": _bf16(wv[fsl].T),
            "woT": _bf16(wo[:, fsl].T),
            "bqc": np.ascontiguousarray(bq[fsl].reshape(4, 128).T, np.float32),
            "strip0": strip0_ones if hg == 0 else strip0_band,
            "supb": supb,
        })
    return in_maps


def kernel(x, wq, bq, wk, bk, wv, bv, wo, bo):
    from concourse.bass_utils import run_bass_kernel_spmd

    x, wq, bq, wk, bk, wv, bv, wo, bo = (
        np.asarray(a, np.float32) for a in (x, wq, bq, wk, bk, wv, bv, wo, bo))

    if "nc" not in _cache:
        _cache["nc"] = _build()
    nc = _cache["nc"]

    in_maps = _host_inputs(x, wq, bq, wk, bk, wv, bv, wo, bo)
    res = run_bass_kernel_spmd(nc, in_maps, core_ids=list(range(NCORES)))
    _cache["last_results"] = res

    const = (bo + bv @ wo.T).astype(np.float32)        # [1024]
    out = np.empty((B, N, E), np.float32)
    for b in range(B):
        out[b] = res.results[2 * b]["out"] + res.results[2 * b + 1]["out"] + const
    return out


# revision 21
# speedup vs baseline: 1.2190x; 1.0521x over previous
"""Local/global multihead attention on 8 NeuronCores (Trainium2, Bass/Tile).

Sharding: core c = b*2 + hg  (b = batch 0..3, hg = head-group 0/1, 8 heads each).
Each core computes q/k/v projections for its 8 heads on its batch, head-local
attention (slot 0 runs a dense 2048-key path driven by a per-core mask so the
SPMD program is uniform: hg0's slot 0 is the true global head with an all-ones
mask, hg1's slot 0 is a local head with a band mask), banded attention with
narrowed tq windows for slots 1-7, and the output projection restricted to its
head-group columns of wo. Host sums the two head-group partials per batch and
adds bo + bv @ wo.T (valid because softmax rows sum to 1).

Performance structure: s (token-quarter) outer loop; banded heads are
processed in even/odd pairs whose K=64 QK matmuls land in PE row-groups 0/64
(tile_position auto-derived from base_partition) and overlap on the 128x128
array; v-projection is emitted just-in-time inside slot0's first pass and the
fc1-3 q/k projections are interleaved between the first quarter's attention
blocks so ScalarE/DVE attention work overlaps PE projection work. Banded mask
multiplies are consolidated to one [128,2304] DVE op per (head, s) via a
block-layout scratch tile. The k bias is dropped (softmax shift invariance)
and the q bias rides the ScalarE PSUM evacuation. Softmax denominators use
reciprocal_approx_fast (SBUF-staged: the op mis-executes on PSUM inputs).

All matmul operands are bf16 (TensorE runs 1 cyc/row vs 4 for fp32); PSUM
accumulation is fp32 throughout.
"""
import numpy as np
import ml_dtypes

E, H, D, LK = 1024, 16, 64, 128
SCALE = D ** -0.5
B, N = 4, 2048
FG = 512          # features per head-group (8 heads * 64)
NCORES = 8

# narrowed tq windows per dj variant (delta = (dj-1)*128)
WIN = [(0, 128), (0, 256), (0, 384), (128, 512), (256, 512), (384, 512)]

_cache = {}


def _bf16(a):
    return np.ascontiguousarray(a.astype(ml_dtypes.bfloat16))


def _build():
    import concourse.bacc as bacc
    import concourse.tile as tile
    import concourse.mybir as mybir
    from concourse.bass import ts

    dt = mybir.dt
    AF = mybir.ActivationFunctionType

    nc = bacc.Bacc("TRN2", target_bir_lowering=False, debug=False,
                   num_devices=NCORES)

    xT = nc.dram_tensor("xT", [E, N], dt.bfloat16, kind="ExternalInput")
    wqT = nc.dram_tensor("wqT", [E, FG], dt.bfloat16, kind="ExternalInput")
    wkT = nc.dram_tensor("wkT", [E, FG], dt.bfloat16, kind="ExternalInput")
    wvT = nc.dram_tensor("wvT", [E, FG], dt.bfloat16, kind="ExternalInput")
    woT = nc.dram_tensor("woT", [FG, E], dt.bfloat16, kind="ExternalInput")
    bqc = nc.dram_tensor("bqc", [128, 4], dt.float32, kind="ExternalInput")
    # strip0 [128, 3968]: slot-0 mask table. slice at 512s-128jc+1920 gives the
    # [128,512] mask for (jc, s): all-ones on hg0 (global head), band on hg1.
    strip0 = nc.dram_tensor("strip0", [128, 3968], dt.bfloat16, kind="ExternalInput")
    # supb [128, 2304]: banded mask blocks; block dj at cols [384dj, 384dj+w).
    supb = nc.dram_tensor("supb", [128, 2304], dt.bfloat16, kind="ExternalInput")
    out = nc.dram_tensor("out", [N, E], dt.float32, kind="ExternalOutput")

    with tile.TileContext(nc) as tc:
        with (
            tc.tile_pool(name="wts", bufs=1) as wts,
            tc.tile_pool(name="xp", bufs=1) as xp,
            tc.tile_pool(name="qkv", bufs=1) as qkv,
            tc.tile_pool(name="att", bufs=3) as att,
            tc.tile_pool(name="sup", bufs=2) as sup,
            tc.tile_pool(name="small", bufs=4) as small,
            tc.tile_pool(name="ps", bufs=6, space="PSUM") as psp,
            tc.tile_pool(name="av", bufs=2, space="PSUM") as avp,
        ):
            # ---- load weights/x/masks ----
            xT_t = [xp.tile([128, N], dt.bfloat16, name=f"xT{i}", tag=f"xT{i}") for i in range(8)]
            wq_t = [wts.tile([128, FG], dt.bfloat16, name=f"wq{i}", tag=f"wq{i}") for i in range(8)]
            wk_t = [wts.tile([128, FG], dt.bfloat16, name=f"wk{i}", tag=f"wk{i}") for i in range(8)]
            wv_t = [wts.tile([128, FG], dt.bfloat16, name=f"wv{i}", tag=f"wv{i}") for i in range(8)]
            # interleave x and q/k weight chunks so the first projection
            # matmuls (accumulating over ec in order) start immediately
            for ec in range(8):
                nc.sync.dma_start(xT_t[ec][:], xT[ts(ec, 128), :])
                nc.sync.dma_start(wq_t[ec][:], wqT[ts(ec, 128), :])
                nc.sync.dma_start(wk_t[ec][:], wkT[ts(ec, 128), :])
            for ec in range(8):
                nc.sync.dma_start(wv_t[ec][:], wvT[ts(ec, 128), :])
            wo_t = [wts.tile([128, E], dt.bfloat16, name=f"wo{i}", tag=f"wo{i}") for i in range(4)]
            for fc in range(4):
                nc.sync.dma_start(wo_t[fc][:], woT[ts(fc, 128), :])
            bq_t = small.tile([128, 4], dt.float32, name="bq", tag="bq")
            nc.sync.dma_start(bq_t[:], bqc[:, :])
            s0_t = wts.tile([128, 3968], dt.bfloat16, name="s0", tag="s0")
            nc.sync.dma_start(s0_t[:], strip0[:, :])
            sup_t = wts.tile([128, 2304], dt.bfloat16, name="supb", tag="supb")
            nc.sync.dma_start(sup_t[:], supb[:, :])

            qT_sb = [qkv.tile([128, N], dt.bfloat16, name=f"qT{i}", tag=f"qT{i}") for i in range(4)]
            kT_sb = [qkv.tile([128, N], dt.bfloat16, name=f"kT{i}", tag=f"kT{i}") for i in range(4)]
            v_sb = [qkv.tile([128, 8 * 72], dt.bfloat16, name=f"v{i}", tag=f"v{i}") for i in range(16)]
            outTn = [qkv.tile([128, N], dt.bfloat16, name=f"outTn{i}", tag=f"outTn{i}") for i in range(4)]

            # k bias dropped (softmax shift invariance); q bias rides the
            # ScalarE evacuation as a per-partition activation bias.
            def emit_qk_tile(fc, tcn):
                for dst, w_t, biased in ((qT_sb, wq_t, True), (kT_sb, wk_t, False)):
                    ps = psp.tile([128, 512], dt.float32, name="ps", tag="ps")
                    for ec in range(8):
                        nc.tensor.matmul(
                            ps[:], w_t[ec][:, ts(fc, 128)],
                            xT_t[ec][:, ts(tcn, 512)],
                            start=(ec == 0), stop=(ec == 7))
                    if biased:
                        nc.scalar.activation(
                            dst[fc][:, ts(tcn, 512)], ps[:], AF.Identity,
                            bias=bq_t[:, fc:fc + 1])
                    else:
                        nc.scalar.copy(dst[fc][:, ts(tcn, 512)], ps[:])

            def emit_v(tcn):
                # v natural layout, per-head 72-col strided tiles w/ ones col
                ps = psp.tile([128, 512], dt.float32, name="ps", tag="ps")
                for ec in range(8):
                    nc.tensor.matmul(ps[:], xT_t[ec][:, ts(tcn, 128)], wv_t[ec][:],
                                     start=(ec == 0), stop=(ec == 7))
                src = ps[:].rearrange("p (h d) -> p h d", h=8)
                dst = v_sb[tcn][:].rearrange("p (h d) -> p h d", h=8)[:, :, 0:64]
                nc.vector.tensor_copy(dst, src)
                ones = v_sb[tcn][:].rearrange("p (h d) -> p h d", h=8)[:, :, 64:65]
                nc.vector.memset(ones, 1.0)

            def head_rows(t, h):
                r0 = (h % 2) * 64
                return t[h // 2][r0:r0 + 64, :]

            def norm(h, s, av):
                # den -> SBUF (approx recip mis-executes on PSUM), recip,
                # broadcast to 64 partitions, scale the head's outTn rows.
                den = small.tile([1, 512], dt.float32, name="den", tag="den")
                nc.scalar.copy(den[:], av[64:65, :])
                rec = small.tile([1, 512], dt.float32, name="rec", tag="rec")
                nc.vector.reciprocal_approx_fast(rec[:], den[:])
                rec64 = small.tile([64, 512], dt.float32, name="rec64", tag="rec64")
                nc.gpsimd.partition_broadcast(rec64[:], rec[:])
                nc.vector.tensor_mul(head_rows(outTn, h)[:, ts(s, 512)],
                                     av[0:64, :], rec64[:])

            def banded(h, s):
                # one banded head for token-quarter s: QK per window into the
                # block-layout at_super, exp per window, ONE mask multiply,
                # then AV accumulation per window.
                qh = head_rows(qT_sb, h)
                kh = head_rows(kT_sb, h)
                djs = [dj for dj in range(6) if 0 <= s * 4 - 1 + dj <= 15]
                ats = sup.tile([128, 2304], dt.bfloat16, name="ats", tag="aats")
                pss = []
                for dj in djs:
                    jc = s * 4 - 1 + dj
                    c0, c1 = WIN[dj]
                    ps = psp.tile([128, 512], dt.float32, name="ps", tag="ps")
                    nc.tensor.matmul(ps[:, c0:c1], kh[:, ts(jc, 128)],
                                     qh[:, s * 512 + c0:s * 512 + c1],
                                     start=True, stop=True)
                    pss.append(ps)
                for dj, ps in zip(djs, pss):
                    c0, c1 = WIN[dj]
                    nc.scalar.activation(ats[:, 384 * dj:384 * dj + (c1 - c0)],
                                         ps[:, c0:c1], AF.Exp, scale=float(SCALE))
                nc.vector.tensor_mul(ats[:], ats[:], sup_t[:])
                av = avp.tile([128, 512], dt.float32, name="av", tag="av")
                for i, dj in enumerate(djs):
                    jc = s * 4 - 1 + dj
                    c0, c1 = WIN[dj]
                    nc.tensor.matmul(
                        av[0:65, c0:c1], v_sb[jc][:, h * 72:h * 72 + 65],
                        ats[:, 384 * dj:384 * dj + (c1 - c0)],
                        start=(i == 0), stop=(i == len(djs) - 1),
                        skip_group_check=True)
                norm(h, s, av)

            def banded_pair(ha, hb, s):
                # even/odd head pair: QK matmuls alternate PE row-groups 0/64
                # (auto tile_position) so the K=64 matmuls run concurrently.
                djs = [dj for dj in range(6) if 0 <= s * 4 - 1 + dj <= 15]
                tiles = {}
                for h, tag in ((ha, "pa"), (hb, "pb")):
                    tiles[h] = (sup.tile([128, 2304], dt.bfloat16, name="ats",
                                         tag=tag + "ats"), [])
                for dj in djs:
                    jc = s * 4 - 1 + dj
                    c0, c1 = WIN[dj]
                    for h in (ha, hb):
                        qh = head_rows(qT_sb, h)
                        kh = head_rows(kT_sb, h)
                        ps = psp.tile([128, 512], dt.float32, name="ps", tag="ps")
                        nc.tensor.matmul(ps[:, c0:c1], kh[:, ts(jc, 128)],
                                         qh[:, s * 512 + c0:s * 512 + c1],
                                         start=True, stop=True)
                        tiles[h][1].append(ps)
                for h in (ha, hb):
                    ats, pss = tiles[h]
                    for dj, ps in zip(djs, pss):
                        c0, c1 = WIN[dj]
                        nc.scalar.activation(ats[:, 384 * dj:384 * dj + (c1 - c0)],
                                             ps[:, c0:c1], AF.Exp,
                                             scale=float(SCALE))
                    nc.vector.tensor_mul(ats[:], ats[:], sup_t[:])
                avs = {}
                for h in (ha, hb):
                    avs[h] = avp.tile([128, 512], dt.float32, name="av",
                                      tag="av")
                for i, dj in enumerate(djs):
                    jc = s * 4 - 1 + dj
                    c0, c1 = WIN[dj]
                    for h in (ha, hb):
                        nc.tensor.matmul(
                            avs[h][0:65, c0:c1],
                            v_sb[jc][:, h * 72:h * 72 + 65],
                            tiles[h][0][:, 384 * dj:384 * dj + (c1 - c0)],
                            start=(i == 0), stop=(i == len(djs) - 1),
                            skip_group_check=True)
                norm(ha, s, avs[ha])
                norm(hb, s, avs[hb])

            def outproj(s):
                for tcn in range(4 * s, 4 * s + 4):
                    for oc in range(2):
                        ps = psp.tile([128, 512], dt.float32, name="ps", tag="ps")
                        for fc in range(4):
                            nc.tensor.matmul(ps[:], outTn[fc][:, ts(tcn, 128)],
                                             wo_t[fc][:, ts(oc, 512)],
                                             start=(fc == 0), stop=(fc == 3))
                        ob = att.tile([128, 512], dt.float32, name="ob", tag="ob")
                        nc.vector.tensor_copy(ob[:], ps[:])
                        nc.sync.dma_start(out[ts(tcn, 128), ts(oc, 512)], ob[:])

            # ---- main schedule ----
            # Projections are pipelined into the attention blocks at
            # (fc, tcn) granularity: block s's banded pairs only touch q/k
            # token tiles tcn <= s+1, so later tiles are emitted just-in-time
            # inside earlier blocks, keeping dense PE work under the
            # ScalarE-bound exp stretches (HAM stays warm). outproj(s) is
            # emitted inside block s+1 for the same reason.
            for tcn in range(4):
                emit_qk_tile(0, tcn)
            qh0 = head_rows(qT_sb, 0)
            kh0 = head_rows(kT_sb, 0)
            for s in range(4):
                # slot 0: dense 2048-key path; mask = static strip0 slices.
                # v projection is emitted just-in-time on the first pass.
                avg = avp.tile([128, 512], dt.float32, name="avg", tag="av")
                for jc in range(16):
                    if s == 0:
                        emit_v(jc)
                    ps = psp.tile([128, 512], dt.float32, name="ps", tag="ps")
                    nc.tensor.matmul(ps[:], kh0[:, ts(jc, 128)],
                                     qh0[:, ts(s, 512)], start=True, stop=True)
                    at = att.tile([128, 512], dt.bfloat16, name="at", tag="at")
                    nc.scalar.activation(at[:], ps[:], AF.Exp, scale=float(SCALE))
                    off = 512 * s - 128 * jc + 1920
                    nc.vector.tensor_mul(at[:], at[:], s0_t[:, off:off + 512])
                    nc.tensor.matmul(
                        avg[0:65, :], v_sb[jc][:, 0:65],
                        at[:], start=(jc == 0), stop=(jc == 15),
                        skip_group_check=True)
                norm(0, s, avg)
                if s > 0:
                    outproj(s - 1)
                banded(1, s)
                for pi, (ha, hb) in enumerate(((2, 3), (4, 5), (6, 7))):
                    fc = pi + 1
                    if s == 0:
                        emit_qk_tile(fc, 0)
                        emit_qk_tile(fc, 1)
                    elif s < 3:
                        emit_qk_tile(fc, s + 1)
                    banded_pair(ha, hb, s)
            outproj(3)
    nc.finalize()
    return nc


def _host_inputs(x, wq, bq, wk, bk, wv, bv, wo, bo):
    """Build the 8 per-core input dicts."""
    r = np.arange(128)[:, None]
    # strip0 [128, 3968]: band at |r + 1920 - c| <= LK (hg1) or all ones (hg0)
    c = np.arange(3968)[None, :]
    strip0_band = _bf16((np.abs(r + 1920 - c) <= LK).astype(np.float32))
    strip0_ones = _bf16(np.ones((128, 3968), np.float32))
    # supb [128, 2304]: block dj holds the window-(dj) band mask at 384*dj
    supb = np.zeros((128, 2304), np.float32)
    for dj in range(6):
        c0, c1 = WIN[dj]
        cc = np.arange(c0, c1)[None, :]
        supb[:, 384 * dj:384 * dj + (c1 - c0)] = (
            np.abs((dj - 1) * 128 + r - cc) <= LK)
    supb = _bf16(supb)

    in_maps = []
    for core in range(NCORES):
        b, hg = core // 2, core % 2
        fsl = slice(hg * FG, (hg + 1) * FG)
        in_maps.append({
            "xT": _bf16(x[b].T),
            "wqT": _bf16(wq[fsl].T),
            "wkT": _bf16(wk[fsl].T),
            "wvT": _bf16(wv[fsl].T),
            "woT": _bf16(wo[:, fsl].T),
            "bqc": np.ascontiguousarray(bq[fsl].reshape(4, 128).T, np.float32),
            "strip0": strip0_ones if hg == 0 else strip0_band,
            "supb": supb,
        })
    return in_maps


def kernel(x, wq, bq, wk, bk, wv, bv, wo, bo):
    from concourse.bass_utils import run_bass_kernel_spmd

    x, wq, bq, wk, bk, wv, bv, wo, bo = (
        np.asarray(a, np.float32) for a in (x, wq, bq, wk, bk, wv, bv, wo, bo))

    if "nc" not in _cache:
        _cache["nc"] = _build()
    nc = _cache["nc"]

    in_maps = _host_inputs(x, wq, bq, wk, bk, wv, bv, wo, bo)
    res = run_bass_kernel_spmd(nc, in_maps, core_ids=list(range(NCORES)))
    _cache["last_results"] = res

    const = (bo + bv @ wo.T).astype(np.float32)        # [1024]
    out = np.empty((B, N, E), np.float32)
    for b in range(B):
        out[b] = res.results[2 * b]["out"] + res.results[2 * b + 1]["out"] + const
    return out
